# revision 13
# baseline (speedup 1.0000x reference)
"""Trainium2 Bass kernel for nn_Decoder (GNN message passing + LSTM).

Strategy (8 NeuronCores, SPMD):
  - Graph nodes are partitioned into 8 contiguous ranges (6250/core); each
    core owns the aggregation + dense math for its nodes. Within a core,
    nodes are sorted by in-degree (descending) so that per-128-node blocks
    have near-uniform degree; each block's incoming messages are laid out as
    a dense padded [128 nodes, K_b slots] grid (halo materialization of the
    edge-cut, built host-side as pure index gathers).
  - GraphConv is computed aggregate-first: because the aggregation commutes
    with the linear layer, the device sums raw (norm-scaled) neighbor
    features with a single strided DVE reduction per block, then applies the
    weight matrix with one matmul per block and fuses the in-norm scaling
    into the Gelu activation at PSUM eviction.
  - The LSTM branch is data-parallel over batch (4 sequences/core) in a
    transposed layout (hidden on partitions) so the per-step elementwise ops
    use all 128 lanes; gate projections of the inputs are precomputed as
    large matmuls; the recurrent matmul runs on the PE with the four gate
    chunks of W_hh. The LSTM is split across the two launches (state is
    carried through) so it overlaps with both GNN layers' DMA traffic.
  - Two launches: layer-2's input is the gelu output of layer 1, which must
    be re-expanded along edges between launches (host-side index gather).
"""

import sys

sys.path.insert(0, "/opt/trn_rl_repo")

import numpy as np

import concourse.bacc as bacc
import concourse.bass as bass
import concourse.mybir as mybir
from concourse.tile import TileContext
from concourse import bass_utils
from concourse.masks import make_identity

F32 = mybir.dt.float32
F16 = mybir.dt.float16

# problem constants (hardcoded per contract)
N = 50000
NC = 8
NPC = N // NC  # 6250 nodes per core
PADN = 6272  # 49 * 128
NBLK = PADN // 128  # 49
NTIL = (NBLK + 1) // 2  # 25 (L2 block pairs)
D = 64
F = 128  # TD + SCD
B, T = 32, 512
BPC = B // NC  # 4 sequences per core
OT = 128
TSPLIT = 256  # LSTM steps done in launch 1
INTERLEAVE = True  # emit GNN pieces between LSTM steps


# --------------------------------------------------------------------------
# host-side graph prep (index metadata + halo materialization layouts)
# --------------------------------------------------------------------------


def _prep_graph(src, dst):
    src = np.asarray(src).astype(np.int64).ravel()
    dst = np.asarray(dst).astype(np.int64).ravel()
    deg_out = np.bincount(src, minlength=N)
    deg_in = np.bincount(dst, minlength=N)
    n_out = np.maximum(deg_out, 1).astype(np.float32) ** -0.5
    n_in = np.maximum(deg_in, 1).astype(np.float32) ** -0.5

    perms = []
    for c in range(NC):
        nodes = np.arange(c * NPC, (c + 1) * NPC)
        order = np.argsort(-deg_in[nodes], kind="stable")
        perms.append(nodes[order])

    # per-block slot count K[b]: max over cores of the block's max in-degree
    K = np.zeros(NBLK, dtype=np.int64)
    for c in range(NC):
        dsort = deg_in[perms[c]]
        for b in range(NBLK):
            lo = b * 128
            if lo < NPC:
                K[b] = max(K[b], int(dsort[lo]))
    colbase = np.zeros(NBLK + 1, dtype=np.int64)
    colbase[1:] = np.cumsum(128 * K)
    COLS1 = int(colbase[-1])

    # L2 tile pairs (two blocks stacked on partition halves)
    K2 = np.zeros(NTIL, dtype=np.int64)
    for t in range(NTIL):
        K2[t] = K[2 * t]
        if 2 * t + 1 < NBLK:
            K2[t] = max(K2[t], K[2 * t + 1])
    tilebase = np.zeros(NTIL + 1, dtype=np.int64)
    tilebase[1:] = np.cumsum(128 * K2)
    COLS2 = int(tilebase[-1])

    # per-core edge -> slot mapping
    edge = []
    owner = dst // NPC
    for c in range(NC):
        rank = np.empty(NPC, dtype=np.int64)
        rank[perms[c] - c * NPC] = np.arange(NPC)
        m = owner == c
        es = src[m]
        r = rank[dst[m] - c * NPC]
        o = np.argsort(r, kind="stable")
        es = es[o]
        r = r[o]
        starts = np.zeros(NPC, dtype=np.int64)
        cnt = np.bincount(r, minlength=NPC)
        starts[1:] = np.cumsum(cnt)[:-1]
        k = np.arange(r.shape[0]) - starts[r]
        b = r // 128
        n = r % 128
        col1 = colbase[b] + n * K[b] + k
        tt = b // 2
        col2 = tilebase[tt] + n * K2[tt] + k
        hi = (b % 2).astype(bool)
        edge.append((es, col1, col2, hi))

    # per-core in-norm arranged [128, NBLK] (col b = block b)
    nin_core = []
    for c in range(NC):
        v = np.ones(PADN, dtype=np.float32)
        v[:NPC] = n_in[perms[c]]
        nin_core.append(np.ascontiguousarray(v.reshape(NBLK, 128).T))

    return dict(
        n_out=n_out,
        perms=perms,
        K=K,
        colbase=colbase,
        COLS1=COLS1,
        K2=K2,
        tilebase=tilebase,
        COLS2=COLS2,
        edge=edge,
        nin_core=nin_core,
    )


def _expand_l1(g, x_cat):
    """x_cat [N, 128] -> per-core X1ET [128, COLS1] (feature-major slots)."""
    y1T = np.ascontiguousarray((x_cat * g["n_out"][:, None]).T.astype(np.float16))
    out = []
    for c in range(NC):
        es, col1, _, _ = g["edge"][c]
        X = np.zeros((F, g["COLS1"]), dtype=np.float16)
        X[:, col1] = y1T[:, es]
        out.append(X)
    return out


def _expand_l2(g, out1_g):
    """out1_g [N, 64] -> per-core X2ET [128, COLS2] (two blocks stacked)."""
    y2T = np.ascontiguousarray((out1_g * g["n_out"][:, None]).T.astype(np.float16))
    out = []
    for c in range(NC):
        es, _, col2, hi = g["edge"][c]
        X = np.zeros((128, g["COLS2"]), dtype=np.float16)
        lo = ~hi
        X[:D, col2[lo]] = y2T[:, es[lo]]
        X[D:, col2[hi]] = y2T[:, es[hi]]
        out.append(X)
    return out


# --------------------------------------------------------------------------
# device programs
# --------------------------------------------------------------------------

ActFn = mybir.ActivationFunctionType
SCH = 6  # slots per PE accumulation piece (keeps PE bursts short)




def _emit_xproj(nc, cpool, xpool, ppool, xt, wih, biast, t0, t1):
    """xprojT [128, 16*(t1-t0)] with columns (t, j, b); bias folded in."""
    nsteps = t1 - t0
    xprojT = xpool.tile([128, 16 * nsteps], F16, tag="xprojT")
    xp5 = xprojT[:].rearrange(
        "p (c t j b) -> p c j t b", c=nsteps // 128, t=128, j=4, b=4
    )
    for ci in range(nsteps // 128):
        for j in range(4):
            px = ppool.tile([128, 512], F32, tag="px")
            nc.tensor.matmul(
                out=px[:],
                lhsT=wih[:, 128 * j : 128 * (j + 1)],
                rhs=xt[:, (t0 + ci * 128) * 4 : (t0 + (ci + 1) * 128) * 4],
                start=True,
                stop=True,
            )
            nc.scalar.activation(
                out=xp5[:, ci, j],
                in_=px[:].rearrange("p (t b) -> p t b", t=128, b=4),
                func=ActFn.Identity,
                bias=biast[:, j : j + 1],
            )
    return xprojT


def _emit_lstm(nc, spool, ppool, whh, idm, xprojT, hbuf, c_tile, nsteps, pieces=None):
    """Gate layout is host-permuted to (i, f, o, g). hbuf/whh are fp16.

    `pieces` is a list of thunks emitting GNN work; they are interleaved
    between step emissions so the Tile scheduler gives the serial LSTM
    chain priority and the GNN fills engine idle time.
    """
    AO = mybir.AluOpType
    pieces = pieces or []
    if not INTERLEAVE:
        for p in pieces:
            p()
        pieces = []
    done = 0
    for t in range(nsteps):
        want = (t + 1) * len(pieces) // nsteps
        while done < want:
            pieces[done]()
            done += 1
        pg = ppool.tile([128, 16], F32, tag="pg")
        h_prev = hbuf[:, t * 4 : (t + 1) * 4]
        # xproj folded in via identity matmul: no h dependency, so it runs
        # during the previous step's elementwise chain
        nc.tensor.matmul(
            out=pg[:],
            lhsT=idm[:],
            rhs=xprojT[:, 16 * t : 16 * (t + 1)],
            start=True,
            stop=False,
            skip_group_check=True,
        )
        # g-gate chunk first so tanh(g) can overlap the i/f/o matmuls
        for j in (3, 0, 1, 2):
            nc.tensor.matmul(
                out=pg[:, 4 * j : 4 * (j + 1)],
                lhsT=whh[:, 128 * j : 128 * (j + 1)],
                rhs=h_prev,
                start=False,
                stop=True,
                skip_group_check=True,
            )
        tg = spool.tile([128, 4], F32, tag="tg")
        nc.scalar.activation(tg[:], pg[:, 12:16], ActFn.Tanh)
        sig = spool.tile([128, 12], F32, tag="sig")
        nc.scalar.activation(sig[:], pg[:, 0:12], ActFn.Sigmoid)
        ig = spool.tile([128, 4], F32, tag="ig")
        nc.vector.tensor_tensor(out=ig[:], in0=sig[:, 0:4], in1=tg[:], op=AO.mult)
        nc.vector.tensor_tensor(out=c_tile[:], in0=c_tile[:], in1=sig[:, 4:8], op=AO.mult)
        nc.vector.tensor_tensor(out=c_tile[:], in0=c_tile[:], in1=ig[:], op=AO.add)
        tcl = spool.tile([128, 4], F32, tag="tcl")
        nc.scalar.activation(tcl[:], c_tile[:], ActFn.Tanh)
        nc.vector.tensor_tensor(
            out=hbuf[:, (t + 1) * 4 : (t + 2) * 4], in0=sig[:, 8:12], in1=tcl[:], op=AO.mult
        )


def _gnn_block_pieces(nc, gpool, kb, dma_emit, mm_emit, fin_emit):
    """Thunks for one block: DMA load, slot-matmul chunks (PSUM-accumulated),
    then eviction/transpose/scale."""
    st = {}
    thunks = [lambda: dma_emit(st)]
    nch = (kb + SCH - 1) // SCH
    for q in range(nch):
        k0, k1 = q * SCH, min(kb, (q + 1) * SCH)
        thunks.append(lambda k0=k0, k1=k1: mm_emit(st, k0, k1, k1 == kb))
    thunks.append(lambda: fin_emit(st))
    return thunks


def _build_l1(g):
    K = g["K"]
    colbase = g["colbase"]
    COLS1 = g["COLS1"]
    nc = bacc.Bacc("TRN2", target_bir_lowering=False, debug=False, num_devices=NC)
    x1e = nc.dram_tensor("x1e", [F, COLS1], F16, kind="ExternalInput")
    xt = nc.dram_tensor("xt", [F, T * BPC], F32, kind="ExternalInput")
    wih = nc.dram_tensor("wih", [F, 4 * OT], F32, kind="ExternalInput")
    whh = nc.dram_tensor("whh", [OT, 4 * OT], F16, kind="ExternalInput")
    biast = nc.dram_tensor("biast", [OT, 4], F32, kind="ExternalInput")
    w1 = nc.dram_tensor("w1", [F, D], F16, kind="ExternalInput")
    nin = nc.dram_tensor("nin", [128, NBLK], F32, kind="ExternalInput")
    idm = nc.dram_tensor("idm", [128, 128], F16, kind="ExternalInput")
    out1 = nc.dram_tensor("out1", [PADN, D], F32, kind="ExternalOutput")
    hbufo = nc.dram_tensor("hbufo", [128, 4 * TSPLIT], F16, kind="ExternalOutput")
    ho = nc.dram_tensor("ho", [128, 4], F16, kind="ExternalOutput")
    co = nc.dram_tensor("co", [128, 4], F32, kind="ExternalOutput")

    with TileContext(nc) as tc:
        with (
            tc.tile_pool(name="consts", bufs=1) as cpool,
            tc.tile_pool(name="xproj", bufs=1) as xpool,
            tc.tile_pool(name="state", bufs=1) as stpool,
            tc.tile_pool(name="small", bufs=3) as spool,
            tc.tile_pool(name="gnn", bufs=3) as gpool,
            tc.tile_pool(name="psumx", bufs=2, space="PSUM") as ppoolx,
            tc.tile_pool(name="psumg", bufs=2, space="PSUM") as ppoolg,
            tc.tile_pool(name="psumz", bufs=2, space="PSUM") as ppoolz,
            tc.tile_pool(name="psumt", bufs=2, space="PSUM") as ppoolt,
        ):
            xt_t = cpool.tile([F, T * BPC], F32)
            nc.sync.dma_start(out=xt_t[:], in_=xt[:])
            wih_t = cpool.tile([F, 4 * OT], F32)
            nc.sync.dma_start(out=wih_t[:], in_=wih[:])
            whh_t = cpool.tile([OT, 4 * OT], F16)
            nc.sync.dma_start(out=whh_t[:], in_=whh[:])
            biast_t = cpool.tile([OT, 4], F32)
            nc.sync.dma_start(out=biast_t[:], in_=biast[:])
            w1_t = cpool.tile([F, D], F16)
            nc.sync.dma_start(out=w1_t[:], in_=w1[:])
            nin_t = cpool.tile([128, NBLK], F32)
            nc.sync.dma_start(out=nin_t[:], in_=nin[:])
            idm_t = cpool.tile([128, 128], F16)
            nc.sync.dma_start(out=idm_t[:], in_=idm[:])

            # ---- GNN layer 1 as interleaved pieces: the slot reduction is
            #      folded into the PE matmul (W1 stationary, slot slices
            #      streamed, PSUM-accumulated), so the DVE stays free for the
            #      LSTM chain. gelu batched at the end (ACT table thrash). ----
            idy = cpool.tile([64, 64], F32)
            make_identity(nc, idy[:])
            stage = xpool.tile([128, NBLK * D], F32, tag="stage")
            pieces = []
            for b in range(NBLK):
                if K[b] == 0:
                    pieces.append(
                        lambda b=b: nc.vector.memset(
                            stage[:, b * D : (b + 1) * D], 0.0
                        )
                    )
                    continue
                kb = int(K[b])

                def dma_emit(st, b=b, kb=kb):
                    blk = gpool.tile([128, 128 * kb], F16, tag="blk")
                    st["blk"] = blk
                    nc.sync.dma_start(
                        out=blk[:],
                        in_=x1e[:, int(colbase[b]) : int(colbase[b + 1])],
                    )

                def mm_emit(st, k0, k1, last, kb=kb):
                    if k0 == 0:
                        st["pzt"] = ppoolt.tile([64, 128], F32, tag="pzt", name="pzt")
                    b3 = st["blk"][:].rearrange("p (n k) -> p n k", n=128, k=kb)
                    for k in range(k0, k1):
                        nc.tensor.matmul(
                            out=st["pzt"][:],
                            lhsT=w1_t[:],
                            rhs=b3[:, :, k],
                            start=(k == 0),
                            stop=(k == kb - 1),
                            skip_group_check=True,
                        )

                def fin_emit(st, b=b):
                    zt = spool.tile([64, 128], F32, tag="zt")
                    nc.vector.tensor_copy(out=zt[:], in_=st["pzt"][:])
                    pz = ppoolz.tile([128, D], F32, tag="pz")
                    nc.tensor.transpose(out=pz[:], in_=zt[:], identity=idy[:])
                    nc.vector.tensor_scalar_mul(
                        out=stage[:, b * D : (b + 1) * D],
                        in0=pz[:],
                        scalar1=nin_t[:, b : b + 1],
                    )

                pieces.extend(
                    _gnn_block_pieces(nc, gpool, kb, dma_emit, mm_emit, fin_emit)
                )

            # ---- LSTM first half (GNN pieces fill the gaps) ----
            xprojT = _emit_xproj(nc, cpool, xpool, ppoolx, xt_t, wih_t, biast_t, 0, TSPLIT)
            hbuf = stpool.tile([128, 4 * (TSPLIT + 1)], F16)
            c_tile = stpool.tile([128, 4], F32)
            nc.vector.memset(hbuf[:, 0:4], 0.0)
            nc.vector.memset(c_tile[:], 0.0)
            _emit_lstm(
                nc, spool, ppoolg, whh_t, idm_t, xprojT, hbuf, c_tile, TSPLIT, pieces
            )
            gbuf = xpool.tile([128, NBLK * D], F32, tag="gbuf")
            for gi in range(7):
                sl = slice(gi * 448, (gi + 1) * 448)
                nc.scalar.activation(gbuf[:, sl], stage[:, sl], ActFn.Gelu)
            for b in range(NBLK):
                nc.sync.dma_start(
                    out=out1[b * 128 : (b + 1) * 128, :],
                    in_=gbuf[:, b * D : (b + 1) * D],
                )
            nc.sync.dma_start(out=hbufo[:], in_=hbuf[:, 4 : 4 * (TSPLIT + 1)])
            nc.sync.dma_start(out=ho[:], in_=hbuf[:, 4 * TSPLIT : 4 * (TSPLIT + 1)])
            nc.sync.dma_start(out=co[:], in_=c_tile[:])
    nc.compile()
    return nc


def _build_l2(g):
    K2 = g["K2"]
    tilebase = g["tilebase"]
    COLS2 = g["COLS2"]
    nsteps = T - TSPLIT
    nc = bacc.Bacc("TRN2", target_bir_lowering=False, debug=False, num_devices=NC)
    x2e = nc.dram_tensor("x2e", [128, COLS2], F16, kind="ExternalInput")
    xt = nc.dram_tensor("xt", [F, T * BPC], F32, kind="ExternalInput")
    wih = nc.dram_tensor("wih", [F, 4 * OT], F32, kind="ExternalInput")
    whh = nc.dram_tensor("whh", [OT, 4 * OT], F16, kind="ExternalInput")
    biast = nc.dram_tensor("biast", [OT, 4], F32, kind="ExternalInput")
    w2d = nc.dram_tensor("w2d", [128, D], F16, kind="ExternalInput")
    nin = nc.dram_tensor("nin", [128, NBLK], F32, kind="ExternalInput")
    idm = nc.dram_tensor("idm", [128, 128], F16, kind="ExternalInput")
    hin = nc.dram_tensor("hin", [128, 4], F16, kind="ExternalInput")
    cin = nc.dram_tensor("cin", [128, 4], F32, kind="ExternalInput")
    out2 = nc.dram_tensor("out2", [PADN, D], F32, kind="ExternalOutput")
    hbufo2 = nc.dram_tensor("hbufo2", [128, 4 * nsteps], F16, kind="ExternalOutput")

    with TileContext(nc) as tc:
        with (
            tc.tile_pool(name="consts", bufs=1) as cpool,
            tc.tile_pool(name="xproj", bufs=1) as xpool,
            tc.tile_pool(name="state", bufs=1) as stpool,
            tc.tile_pool(name="small", bufs=3) as spool,
            tc.tile_pool(name="gnn", bufs=3) as gpool,
            tc.tile_pool(name="psumx", bufs=2, space="PSUM") as ppoolx,
            tc.tile_pool(name="psumg", bufs=2, space="PSUM") as ppoolg,
            tc.tile_pool(name="psumz", bufs=2, space="PSUM") as ppoolz,
            tc.tile_pool(name="psumt", bufs=2, space="PSUM") as ppoolt,
        ):
            xt_t = cpool.tile([F, T * BPC], F32)
            nc.sync.dma_start(out=xt_t[:], in_=xt[:])
            wih_t = cpool.tile([F, 4 * OT], F32)
            nc.sync.dma_start(out=wih_t[:], in_=wih[:])
            whh_t = cpool.tile([OT, 4 * OT], F16)
            nc.sync.dma_start(out=whh_t[:], in_=whh[:])
            biast_t = cpool.tile([OT, 4], F32)
            nc.sync.dma_start(out=biast_t[:], in_=biast[:])
            w2d_t = cpool.tile([128, D], F16)
            nc.sync.dma_start(out=w2d_t[:], in_=w2d[:])
            nin_t = cpool.tile([128, NBLK], F32)
            nc.sync.dma_start(out=nin_t[:], in_=nin[:])
            idm_t = cpool.tile([128, 128], F16)
            nc.sync.dma_start(out=idm_t[:], in_=idm[:])

            # ---- GNN layer 2 as interleaved pieces (block pairs stacked on
            #      partition halves; two half-contraction matmul chains) ----
            idy = cpool.tile([64, 64], F32)
            make_identity(nc, idy[:])
            stage = xpool.tile([128, NBLK * D], F32, tag="stage")
            pieces = []
            for t in range(NTIL):
                blocks = [2 * t] + ([2 * t + 1] if 2 * t + 1 < NBLK else [])
                if K2[t] == 0:
                    for b in blocks:
                        pieces.append(
                            lambda b=b: nc.vector.memset(
                                stage[:, b * D : (b + 1) * D], 0.0
                            )
                        )
                    continue
                kt = int(K2[t])

                def dma_emit(st, t=t, kt=kt):
                    blk = gpool.tile([128, 128 * kt], F16, tag="blk")
                    st["blk"] = blk
                    nc.sync.dma_start(
                        out=blk[:],
                        in_=x2e[:, int(tilebase[t]) : int(tilebase[t + 1])],
                    )

                def mm_emit(st, k0, k1, last, kt=kt, blocks=blocks):
                    if k0 == 0:
                        st["pzt"] = [
                            ppoolt.tile([64, 128], F32, tag="pzt", name="pzt")
                            for _ in blocks
                        ]
                    b3 = st["blk"][:].rearrange("p (n k) -> p n k", n=128, k=kt)
                    for k in range(k0, k1):
                        for half in range(len(blocks)):
                            nc.tensor.matmul(
                                out=st["pzt"][half][:],
                                lhsT=w2d_t[half * D : (half + 1) * D, :],
                                rhs=b3[half * D : (half + 1) * D, :, k],
                                start=(k == 0),
                                stop=(k == kt - 1),
                                skip_group_check=True,
                            )

                def fin_emit(st, blocks=blocks):
                    for half, b in enumerate(blocks):
                        zt = spool.tile([64, 128], F32, tag="zt")
                        nc.vector.tensor_copy(out=zt[:], in_=st["pzt"][half][:])
                        pz = ppoolz.tile([128, D], F32, tag="pz")
                        nc.tensor.transpose(out=pz[:], in_=zt[:], identity=idy[:])
                        nc.vector.tensor_scalar_mul(
                            out=stage[:, b * D : (b + 1) * D],
                            in0=pz[:],
                            scalar1=nin_t[:, b : b + 1],
                        )

                pieces.extend(
                    _gnn_block_pieces(nc, gpool, kt, dma_emit, mm_emit, fin_emit)
                )

            # ---- LSTM second half ----
            xprojT = _emit_xproj(
                nc, cpool, xpool, ppoolx, xt_t, wih_t, biast_t, TSPLIT, T
            )
            hbuf = stpool.tile([128, 4 * (nsteps + 1)], F16)
            c_tile = stpool.tile([128, 4], F32)
            hin_t = spool.tile([128, 4], F16, tag="hin")
            nc.sync.dma_start(out=hin_t[:], in_=hin[:])
            nc.vector.tensor_copy(out=hbuf[:, 0:4], in_=hin_t[:])
            cin_t = spool.tile([128, 4], F32, tag="cin")
            nc.sync.dma_start(out=cin_t[:], in_=cin[:])
            nc.vector.tensor_copy(out=c_tile[:], in_=cin_t[:])
            _emit_lstm(
                nc, spool, ppoolg, whh_t, idm_t, xprojT, hbuf, c_tile, nsteps, pieces
            )
            gbuf = xpool.tile([128, NBLK * D], F32, tag="gbuf")
            for gi in range(7):
                sl = slice(gi * 448, (gi + 1) * 448)
                nc.scalar.activation(gbuf[:, sl], stage[:, sl], ActFn.Gelu)
            for b in range(NBLK):
                nc.sync.dma_start(
                    out=out2[b * 128 : (b + 1) * 128, :],
                    in_=gbuf[:, b * D : (b + 1) * D],
                )
            nc.sync.dma_start(out=hbufo2[:], in_=hbuf[:, 4 : 4 * (nsteps + 1)])
    nc.compile()
    return nc


# --------------------------------------------------------------------------
# entry point
# --------------------------------------------------------------------------

_exec_times = []


def kernel(hs, ht, c_hs, c_ht, src, dst, W1, b1, W2, b2, W_ih, W_hh, b_ih, b_hh):
    global _exec_times
    _exec_times = []
    hs = np.asarray(hs, dtype=np.float32)
    ht = np.asarray(ht, dtype=np.float32)
    c_hs = np.asarray(c_hs, dtype=np.float32)
    c_ht = np.asarray(c_ht, dtype=np.float32)
    W1 = np.asarray(W1, dtype=np.float32)
    W2 = np.asarray(W2, dtype=np.float32)
    b1 = np.asarray(b1, dtype=np.float32)
    b2 = np.asarray(b2, dtype=np.float32)
    W_ih = np.asarray(W_ih, dtype=np.float32)
    W_hh = np.asarray(W_hh, dtype=np.float32)
    bias = (np.asarray(b_ih, dtype=np.float32) + np.asarray(b_hh, dtype=np.float32))
    assert np.all(b1 == 0) and np.all(b2 == 0), "nonzero GraphConv bias unsupported"

    g = _prep_graph(src, dst)
    x_cat = np.concatenate([ht, c_hs], axis=1)
    x1e_cores = _expand_l1(g, x_cat)

    # LSTM host layouts
    gperm = np.r_[0:2 * OT, 3 * OT : 4 * OT, 2 * OT : 3 * OT]  # (i,f,g,o)->(i,f,o,g)
    wihT = np.ascontiguousarray(W_ih.T[:, gperm])  # [128 in, 512 gates]
    whhT = np.ascontiguousarray(W_hh.T[:, gperm]).astype(np.float16)
    biastT = np.ascontiguousarray(bias[gperm].reshape(4, OT).T)  # [128, 4]
    w2d = np.concatenate([W2, W2], axis=0).astype(np.float16)  # duplicated halves
    idm = np.eye(128, dtype=np.float16)
    xt_cores = []
    for c in range(NC):
        x = np.concatenate(
            [hs[c * BPC : (c + 1) * BPC], c_ht[c * BPC : (c + 1) * BPC]], axis=2
        )  # [4, 512, 128]
        xt_cores.append(np.ascontiguousarray(x.transpose(2, 1, 0).reshape(F, T * BPC)))

    nc1 = _build_l1(g)
    in_maps1 = [
        {
            "x1e": x1e_cores[c],
            "xt": xt_cores[c],
            "wih": wihT,
            "whh": whhT,
            "biast": biastT,
            "w1": W1.astype(np.float16),
            "nin": g["nin_core"][c],
            "idm": idm,
        }
        for c in range(NC)
    ]
    res1 = bass_utils.run_bass_kernel_spmd(nc1, in_maps1, core_ids=list(range(NC)))
    if res1.exec_time_ns:
        _exec_times.append(res1.exec_time_ns)

    out1_g = np.zeros((N, D), dtype=np.float32)
    for c in range(NC):
        out1_g[g["perms"][c]] = res1.results[c]["out1"][:NPC]
    x2e_cores = _expand_l2(g, out1_g)

    nc2 = _build_l2(g)
    in_maps2 = [
        {
            "x2e": x2e_cores[c],
            "xt": xt_cores[c],
            "wih": wihT,
            "whh": whhT,
            "biast": biastT,
            "w2d": w2d,
            "nin": g["nin_core"][c],
            "idm": idm,
            "hin": res1.results[c]["ho"],
            "cin": res1.results[c]["co"],
        }
        for c in range(NC)
    ]
    res2 = bass_utils.run_bass_kernel_spmd(nc2, in_maps2, core_ids=list(range(NC)))
    if res2.exec_time_ns:
        _exec_times.append(res2.exec_time_ns)

    x_out = np.zeros((N, D), dtype=np.float32)
    for c in range(NC):
        x_out[g["perms"][c]] = res2.results[c]["out2"][:NPC]

    hs_out = np.zeros((B, T, OT), dtype=np.float32)
    for c in range(NC):
        ha = (
            res1.results[c]["hbufo"].astype(np.float32)
            .reshape(OT, TSPLIT, BPC)
            .transpose(2, 1, 0)
        )
        hb = (
            res2.results[c]["hbufo2"].astype(np.float32)
            .reshape(OT, T - TSPLIT, BPC)
            .transpose(2, 1, 0)
        )
        hs_out[c * BPC : (c + 1) * BPC, :TSPLIT] = ha
        hs_out[c * BPC : (c + 1) * BPC, TSPLIT:] = hb
    return hs_out, x_out


# revision 14
# speedup vs baseline: 1.2031x; 1.2031x over previous
"""Trainium2 Bass kernel for nn_Decoder (GNN message passing + LSTM).

Strategy (8 NeuronCores, SPMD):
  - Graph nodes are partitioned into 8 contiguous ranges (6250/core); each
    core owns the aggregation + dense math for its nodes. Within a core,
    nodes are sorted by in-degree (descending) so that per-128-node blocks
    have near-uniform degree; each block's incoming messages are laid out as
    a dense padded [128 nodes, K_b slots] grid (halo materialization of the
    edge-cut, built host-side as pure index gathers).
  - GraphConv is computed aggregate-first: because the aggregation commutes
    with the linear layer, the device sums raw (norm-scaled) neighbor
    features with a single strided DVE reduction per block, then applies the
    weight matrix with one matmul per block and fuses the in-norm scaling
    into the Gelu activation at PSUM eviction.
  - The LSTM branch is data-parallel over batch (4 sequences/core) in a
    transposed layout (hidden on partitions) so the per-step elementwise ops
    use all 128 lanes; gate projections of the inputs are precomputed as
    large matmuls; the recurrent matmul runs on the PE with the four gate
    chunks of W_hh. The LSTM is split across the two launches (state is
    carried through) so it overlaps with both GNN layers' DMA traffic.
  - Two launches: layer-2's input is the gelu output of layer 1, which must
    be re-expanded along edges between launches (host-side index gather).
"""

import sys

sys.path.insert(0, "/opt/trn_rl_repo")

import numpy as np

import concourse.bacc as bacc
import concourse.bass as bass
import concourse.mybir as mybir
from concourse.tile import TileContext
from concourse import bass_utils
from concourse.masks import make_identity

F32 = mybir.dt.float32
F16 = mybir.dt.float16

# problem constants (hardcoded per contract)
N = 50000
NC = 8
NPC = N // NC  # 6250 nodes per core
PADN = 6272  # 49 * 128
NBLK = PADN // 128  # 49
NTIL = (NBLK + 1) // 2  # 25 (L2 block pairs)
D = 64
F = 128  # TD + SCD
B, T = 32, 512
BPC = B // NC  # 4 sequences per core
OT = 128
TSPLIT = 256  # LSTM steps done in launch 1
INTERLEAVE = True  # emit GNN pieces between LSTM steps


# --------------------------------------------------------------------------
# host-side graph prep (index metadata + halo materialization layouts)
# --------------------------------------------------------------------------


def _prep_graph(src, dst):
    src = np.asarray(src).astype(np.int64).ravel()
    dst = np.asarray(dst).astype(np.int64).ravel()
    deg_out = np.bincount(src, minlength=N)
    deg_in = np.bincount(dst, minlength=N)
    n_out = np.maximum(deg_out, 1).astype(np.float32) ** -0.5
    n_in = np.maximum(deg_in, 1).astype(np.float32) ** -0.5

    perms = []
    for c in range(NC):
        nodes = np.arange(c * NPC, (c + 1) * NPC)
        order = np.argsort(-deg_in[nodes], kind="stable")
        perms.append(nodes[order])

    # per-block slot count K[b]: max over cores of the block's max in-degree
    K = np.zeros(NBLK, dtype=np.int64)
    for c in range(NC):
        dsort = deg_in[perms[c]]
        for b in range(NBLK):
            lo = b * 128
            if lo < NPC:
                K[b] = max(K[b], int(dsort[lo]))
    colbase = np.zeros(NBLK + 1, dtype=np.int64)
    colbase[1:] = np.cumsum(128 * K)
    COLS1 = int(colbase[-1])

    # L2 tile pairs (two blocks stacked on partition halves)
    K2 = np.zeros(NTIL, dtype=np.int64)
    for t in range(NTIL):
        K2[t] = K[2 * t]
        if 2 * t + 1 < NBLK:
            K2[t] = max(K2[t], K[2 * t + 1])
    tilebase = np.zeros(NTIL + 1, dtype=np.int64)
    tilebase[1:] = np.cumsum(128 * K2)
    COLS2 = int(tilebase[-1])

    # per-core edge -> slot mapping
    edge = []
    owner = dst // NPC
    for c in range(NC):
        rank = np.empty(NPC, dtype=np.int64)
        rank[perms[c] - c * NPC] = np.arange(NPC)
        m = owner == c
        es = src[m]
        r = rank[dst[m] - c * NPC]
        o = np.argsort(r, kind="stable")
        es = es[o]
        r = r[o]
        starts = np.zeros(NPC, dtype=np.int64)
        cnt = np.bincount(r, minlength=NPC)
        starts[1:] = np.cumsum(cnt)[:-1]
        k = np.arange(r.shape[0]) - starts[r]
        b = r // 128
        n = r % 128
        col1 = colbase[b] + n * K[b] + k
        tt = b // 2
        col2 = tilebase[tt] + n * K2[tt] + k
        hi = (b % 2).astype(bool)
        edge.append((es, col1, col2, hi))

    # per-core in-norm arranged [128, NBLK] (col b = block b)
    nin_core = []
    for c in range(NC):
        v = np.ones(PADN, dtype=np.float32)
        v[:NPC] = n_in[perms[c]]
        nin_core.append(np.ascontiguousarray(v.reshape(NBLK, 128).T))

    return dict(
        n_out=n_out,
        perms=perms,
        K=K,
        colbase=colbase,
        COLS1=COLS1,
        K2=K2,
        tilebase=tilebase,
        COLS2=COLS2,
        edge=edge,
        nin_core=nin_core,
    )


def _expand_l1(g, x_cat):
    """x_cat [N, 128] -> per-core X1ET [128, COLS1] (feature-major slots)."""
    y1T = np.ascontiguousarray((x_cat * g["n_out"][:, None]).T.astype(np.float16))
    out = []
    for c in range(NC):
        es, col1, _, _ = g["edge"][c]
        X = np.zeros((F, g["COLS1"]), dtype=np.float16)
        X[:, col1] = y1T[:, es]
        out.append(X)
    return out


def _expand_l2(g, out1_g):
    """out1_g [N, 64] -> per-core X2ET [128, COLS2] (two blocks stacked)."""
    y2T = np.ascontiguousarray((out1_g * g["n_out"][:, None]).T.astype(np.float16))
    out = []
    for c in range(NC):
        es, _, col2, hi = g["edge"][c]
        X = np.zeros((128, g["COLS2"]), dtype=np.float16)
        lo = ~hi
        X[:D, col2[lo]] = y2T[:, es[lo]]
        X[D:, col2[hi]] = y2T[:, es[hi]]
        out.append(X)
    return out


# --------------------------------------------------------------------------
# device programs
# --------------------------------------------------------------------------

ActFn = mybir.ActivationFunctionType
RCH = 8  # reduction slot-chunk (keeps DVE ops short so the LSTM chain isn't blocked)




def _emit_xproj(nc, cpool, xpool, ppool, xt, wih, biast, t0, t1):
    """xprojT [128, 16*(t1-t0)] with columns (t, j, b); bias folded in."""
    nsteps = t1 - t0
    xprojT = xpool.tile([128, 16 * nsteps], F16, tag="xprojT")
    xp5 = xprojT[:].rearrange(
        "p (c t j b) -> p c j t b", c=nsteps // 128, t=128, j=4, b=4
    )
    for ci in range(nsteps // 128):
        for j in range(4):
            px = ppool.tile([128, 512], F32, tag="px")
            nc.tensor.matmul(
                out=px[:],
                lhsT=wih[:, 128 * j : 128 * (j + 1)],
                rhs=xt[:, (t0 + ci * 128) * 4 : (t0 + (ci + 1) * 128) * 4],
                start=True,
                stop=True,
            )
            nc.scalar.activation(
                out=xp5[:, ci, j],
                in_=px[:].rearrange("p (t b) -> p t b", t=128, b=4),
                func=ActFn.Identity,
                bias=biast[:, j : j + 1],
            )
    return xprojT


def _emit_lstm(nc, spool, ppool, whh, idm, xprojT, hbuf, c_tile, nsteps, pieces=None):
    """Gate layout is host-permuted to (i, f, o, g). hbuf/whh are fp16.

    `pieces` is a list of thunks emitting GNN work; they are interleaved
    between step emissions so the Tile scheduler gives the serial LSTM
    chain priority and the GNN fills engine idle time.
    """
    AO = mybir.AluOpType
    pieces = pieces or []
    if not INTERLEAVE:
        for p in pieces:
            p()
        pieces = []
    done = 0
    for t in range(nsteps):
        want = (t + 1) * len(pieces) // nsteps
        while done < want:
            pieces[done]()
            done += 1
        pg = ppool.tile([128, 16], F32, tag="pg")
        h_prev = hbuf[:, t * 4 : (t + 1) * 4]
        # xproj folded in via identity matmul: no h dependency, so it runs
        # during the previous step's elementwise chain
        nc.tensor.matmul(
            out=pg[:],
            lhsT=idm[:],
            rhs=xprojT[:, 16 * t : 16 * (t + 1)],
            start=True,
            stop=False,
            skip_group_check=True,
        )
        # g-gate chunk first so tanh(g) can overlap the i/f/o matmuls
        for j in (3, 0, 1, 2):
            nc.tensor.matmul(
                out=pg[:, 4 * j : 4 * (j + 1)],
                lhsT=whh[:, 128 * j : 128 * (j + 1)],
                rhs=h_prev,
                start=False,
                stop=True,
                skip_group_check=True,
            )
        tg = spool.tile([128, 4], F32, tag="tg")
        nc.scalar.activation(tg[:], pg[:, 12:16], ActFn.Tanh)
        sig = spool.tile([128, 12], F32, tag="sig")
        nc.scalar.activation(sig[:], pg[:, 0:12], ActFn.Sigmoid)
        ig = spool.tile([128, 4], F32, tag="ig")
        nc.vector.tensor_tensor(out=ig[:], in0=sig[:, 0:4], in1=tg[:], op=AO.mult)
        nc.vector.tensor_tensor(out=c_tile[:], in0=c_tile[:], in1=sig[:, 4:8], op=AO.mult)
        nc.vector.tensor_tensor(out=c_tile[:], in0=c_tile[:], in1=ig[:], op=AO.add)
        tcl = spool.tile([128, 4], F32, tag="tcl")
        nc.scalar.activation(tcl[:], c_tile[:], ActFn.Tanh)
        nc.vector.tensor_tensor(
            out=hbuf[:, (t + 1) * 4 : (t + 2) * 4], in0=sig[:, 8:12], in1=tcl[:], op=AO.mult
        )


def _gnn_block_pieces(nc, rpool, gpool, kb, dma_emit, fin_emit):
    """Thunks for one block: DMA load (fp16), chunked DVE reduces (fp32
    accumulate), combine + matmul + scale."""
    AO = mybir.AluOpType
    st = {}
    thunks = []

    def t_dma():
        blk = gpool.tile([128, 128 * kb], F16, tag="blk", name="blk")
        st["blk"] = blk
        dma_emit(blk)

    thunks.append(t_dma)
    nch = (kb + RCH - 1) // RCH

    def mk_red(q):
        def f():
            b3 = st["blk"][:].rearrange("p (n k) -> p n k", n=128, k=kb)
            k0, k1 = q * RCH, min(kb, (q + 1) * RCH)
            if q == 0:
                aggT = rpool.tile([128, 128], F32, tag="aggT", name="aggT")
                st["aggT"] = aggT
                st["parts"] = []
                tgt = aggT[:]
            else:
                r = rpool.tile([128, 128], F32, tag="rpart", name="rpart")
                st["parts"].append(r)
                tgt = r[:]
            nc.vector.reduce_sum(
                out=tgt, in_=b3[:, :, k0:k1], axis=mybir.AxisListType.X
            )

        return f

    for q in range(nch):
        thunks.append(mk_red(q))

    def t_fin():
        aggT = st["aggT"]
        for r in st["parts"]:
            nc.vector.tensor_tensor(out=aggT[:], in0=aggT[:], in1=r[:], op=AO.add)
        fin_emit(aggT)

    thunks.append(t_fin)
    return thunks


def _build_l1(g):
    K = g["K"]
    colbase = g["colbase"]
    COLS1 = g["COLS1"]
    nc = bacc.Bacc("TRN2", target_bir_lowering=False, debug=False, num_devices=NC)
    x1e = nc.dram_tensor("x1e", [F, COLS1], F16, kind="ExternalInput")
    xt = nc.dram_tensor("xt", [F, T * BPC], F32, kind="ExternalInput")
    wih = nc.dram_tensor("wih", [F, 4 * OT], F32, kind="ExternalInput")
    whh = nc.dram_tensor("whh", [OT, 4 * OT], F16, kind="ExternalInput")
    biast = nc.dram_tensor("biast", [OT, 4], F32, kind="ExternalInput")
    w1 = nc.dram_tensor("w1", [F, D], F32, kind="ExternalInput")
    nin = nc.dram_tensor("nin", [128, NBLK], F32, kind="ExternalInput")
    idm = nc.dram_tensor("idm", [128, 128], F16, kind="ExternalInput")
    out1 = nc.dram_tensor("out1", [PADN, D], F32, kind="ExternalOutput")
    hbufo = nc.dram_tensor("hbufo", [128, 4 * TSPLIT], F16, kind="ExternalOutput")
    ho = nc.dram_tensor("ho", [128, 4], F16, kind="ExternalOutput")
    co = nc.dram_tensor("co", [128, 4], F32, kind="ExternalOutput")

    with TileContext(nc) as tc:
        with (
            tc.tile_pool(name="consts", bufs=1) as cpool,
            tc.tile_pool(name="xproj", bufs=1) as xpool,
            tc.tile_pool(name="state", bufs=1) as stpool,
            tc.tile_pool(name="small", bufs=3) as spool,
            tc.tile_pool(name="red", bufs=8) as rpool,
            tc.tile_pool(name="gnn", bufs=3) as gpool,
            tc.tile_pool(name="psumx", bufs=2, space="PSUM") as ppoolx,
            tc.tile_pool(name="psumg", bufs=2, space="PSUM") as ppoolg,
            tc.tile_pool(name="psumz", bufs=2, space="PSUM") as ppoolz,
        ):
            xt_t = cpool.tile([F, T * BPC], F32)
            nc.sync.dma_start(out=xt_t[:], in_=xt[:])
            wih_t = cpool.tile([F, 4 * OT], F32)
            nc.sync.dma_start(out=wih_t[:], in_=wih[:])
            whh_t = cpool.tile([OT, 4 * OT], F16)
            nc.sync.dma_start(out=whh_t[:], in_=whh[:])
            biast_t = cpool.tile([OT, 4], F32)
            nc.sync.dma_start(out=biast_t[:], in_=biast[:])
            w1_t = cpool.tile([F, D], F32)
            nc.sync.dma_start(out=w1_t[:], in_=w1[:])
            nin_t = cpool.tile([128, NBLK], F32)
            nc.sync.dma_start(out=nin_t[:], in_=nin[:])
            idm_t = cpool.tile([128, 128], F16)
            nc.sync.dma_start(out=idm_t[:], in_=idm[:])

            # ---- GNN layer 1 as interleaved pieces (fp16 inputs, chunked
            #      fp32 DVE reduce; gelu batched at the end) ----
            stage = xpool.tile([128, NBLK * D], F32, tag="stage")
            pieces = []
            for b in range(NBLK):
                if K[b] == 0:
                    pieces.append(
                        lambda b=b: nc.vector.memset(
                            stage[:, b * D : (b + 1) * D], 0.0
                        )
                    )
                    continue

                def dma_emit(blk, b=b):
                    nc.sync.dma_start(
                        out=blk[:],
                        in_=x1e[:, int(colbase[b]) : int(colbase[b + 1])],
                    )

                def fin_emit(aggT, b=b):
                    pz = ppoolz.tile([128, D], F32, tag="pz", name="pz")
                    nc.tensor.matmul(
                        out=pz[:], lhsT=aggT[:], rhs=w1_t[:], start=True, stop=True
                    )
                    nc.vector.tensor_scalar_mul(
                        out=stage[:, b * D : (b + 1) * D],
                        in0=pz[:],
                        scalar1=nin_t[:, b : b + 1],
                    )

                pieces.extend(
                    _gnn_block_pieces(nc, rpool, gpool, int(K[b]), dma_emit, fin_emit)
                )

            # ---- LSTM first half (GNN pieces fill the gaps) ----
            xprojT = _emit_xproj(nc, cpool, xpool, ppoolx, xt_t, wih_t, biast_t, 0, TSPLIT)
            hbuf = stpool.tile([128, 4 * (TSPLIT + 1)], F16)
            c_tile = stpool.tile([128, 4], F32)
            nc.vector.memset(hbuf[:, 0:4], 0.0)
            nc.vector.memset(c_tile[:], 0.0)
            _emit_lstm(
                nc, spool, ppoolg, whh_t, idm_t, xprojT, hbuf, c_tile, TSPLIT, pieces
            )
            gbuf = xpool.tile([128, NBLK * D], F32, tag="gbuf")
            for gi in range(7):
                sl = slice(gi * 448, (gi + 1) * 448)
                nc.scalar.activation(gbuf[:, sl], stage[:, sl], ActFn.Gelu)
            for b in range(NBLK):
                nc.sync.dma_start(
                    out=out1[b * 128 : (b + 1) * 128, :],
                    in_=gbuf[:, b * D : (b + 1) * D],
                )
            nc.sync.dma_start(out=hbufo[:], in_=hbuf[:, 4 : 4 * (TSPLIT + 1)])
            nc.sync.dma_start(out=ho[:], in_=hbuf[:, 4 * TSPLIT : 4 * (TSPLIT + 1)])
            nc.sync.dma_start(out=co[:], in_=c_tile[:])
    nc.compile()
    return nc


def _build_l2(g):
    K2 = g["K2"]
    tilebase = g["tilebase"]
    COLS2 = g["COLS2"]
    nsteps = T - TSPLIT
    nc = bacc.Bacc("TRN2", target_bir_lowering=False, debug=False, num_devices=NC)
    x2e = nc.dram_tensor("x2e", [128, COLS2], F16, kind="ExternalInput")
    xt = nc.dram_tensor("xt", [F, T * BPC], F32, kind="ExternalInput")
    wih = nc.dram_tensor("wih", [F, 4 * OT], F32, kind="ExternalInput")
    whh = nc.dram_tensor("whh", [OT, 4 * OT], F16, kind="ExternalInput")
    biast = nc.dram_tensor("biast", [OT, 4], F32, kind="ExternalInput")
    w2d = nc.dram_tensor("w2d", [128, D], F32, kind="ExternalInput")
    nin = nc.dram_tensor("nin", [128, NBLK], F32, kind="ExternalInput")
    idm = nc.dram_tensor("idm", [128, 128], F16, kind="ExternalInput")
    hin = nc.dram_tensor("hin", [128, 4], F16, kind="ExternalInput")
    cin = nc.dram_tensor("cin", [128, 4], F32, kind="ExternalInput")
    out2 = nc.dram_tensor("out2", [PADN, D], F32, kind="ExternalOutput")
    hbufo2 = nc.dram_tensor("hbufo2", [128, 4 * nsteps], F16, kind="ExternalOutput")

    with TileContext(nc) as tc:
        with (
            tc.tile_pool(name="consts", bufs=1) as cpool,
            tc.tile_pool(name="xproj", bufs=1) as xpool,
            tc.tile_pool(name="state", bufs=1) as stpool,
            tc.tile_pool(name="small", bufs=3) as spool,
            tc.tile_pool(name="red", bufs=8) as rpool,
            tc.tile_pool(name="gnn", bufs=3) as gpool,
            tc.tile_pool(name="psumx", bufs=2, space="PSUM") as ppoolx,
            tc.tile_pool(name="psumg", bufs=2, space="PSUM") as ppoolg,
            tc.tile_pool(name="psumz", bufs=2, space="PSUM") as ppoolz,
        ):
            xt_t = cpool.tile([F, T * BPC], F32)
            nc.sync.dma_start(out=xt_t[:], in_=xt[:])
            wih_t = cpool.tile([F, 4 * OT], F32)
            nc.sync.dma_start(out=wih_t[:], in_=wih[:])
            whh_t = cpool.tile([OT, 4 * OT], F16)
            nc.sync.dma_start(out=whh_t[:], in_=whh[:])
            biast_t = cpool.tile([OT, 4], F32)
            nc.sync.dma_start(out=biast_t[:], in_=biast[:])
            w2d_t = cpool.tile([128, D], F32)
            nc.sync.dma_start(out=w2d_t[:], in_=w2d[:])
            nin_t = cpool.tile([128, NBLK], F32)
            nc.sync.dma_start(out=nin_t[:], in_=nin[:])
            idm_t = cpool.tile([128, 128], F16)
            nc.sync.dma_start(out=idm_t[:], in_=idm[:])

            # ---- GNN layer 2 as interleaved pieces (block pairs stacked) ----
            stage = xpool.tile([128, NBLK * D], F32, tag="stage")
            pieces = []
            for t in range(NTIL):
                blocks = [2 * t] + ([2 * t + 1] if 2 * t + 1 < NBLK else [])
                if K2[t] == 0:
                    for b in blocks:
                        pieces.append(
                            lambda b=b: nc.vector.memset(
                                stage[:, b * D : (b + 1) * D], 0.0
                            )
                        )
                    continue

                def dma_emit(blk, t=t):
                    nc.sync.dma_start(
                        out=blk[:],
                        in_=x2e[:, int(tilebase[t]) : int(tilebase[t + 1])],
                    )

                def fin_emit(aggT, blocks=blocks):
                    for half, b in enumerate(blocks):
                        pz = ppoolz.tile([128, D], F32, tag="pz", name="pz")
                        nc.tensor.matmul(
                            out=pz[:],
                            lhsT=aggT[half * D : (half + 1) * D, :],
                            rhs=w2d_t[half * D : (half + 1) * D, :],
                            start=True,
                            stop=True,
                        )
                        nc.vector.tensor_scalar_mul(
                            out=stage[:, b * D : (b + 1) * D],
                            in0=pz[:],
                            scalar1=nin_t[:, b : b + 1],
                        )

                pieces.extend(
                    _gnn_block_pieces(nc, rpool, gpool, int(K2[t]), dma_emit, fin_emit)
                )

            # ---- LSTM second half ----
            xprojT = _emit_xproj(
                nc, cpool, xpool, ppoolx, xt_t, wih_t, biast_t, TSPLIT, T
            )
            hbuf = stpool.tile([128, 4 * (nsteps + 1)], F16)
            c_tile = stpool.tile([128, 4], F32)
            hin_t = spool.tile([128, 4], F16, tag="hin")
            nc.sync.dma_start(out=hin_t[:], in_=hin[:])
            nc.vector.tensor_copy(out=hbuf[:, 0:4], in_=hin_t[:])
            cin_t = spool.tile([128, 4], F32, tag="cin")
            nc.sync.dma_start(out=cin_t[:], in_=cin[:])
            nc.vector.tensor_copy(out=c_tile[:], in_=cin_t[:])
            _emit_lstm(
                nc, spool, ppoolg, whh_t, idm_t, xprojT, hbuf, c_tile, nsteps, pieces
            )
            gbuf = xpool.tile([128, NBLK * D], F32, tag="gbuf")
            for gi in range(7):
                sl = slice(gi * 448, (gi + 1) * 448)
                nc.scalar.activation(gbuf[:, sl], stage[:, sl], ActFn.Gelu)
            for b in range(NBLK):
                nc.sync.dma_start(
                    out=out2[b * 128 : (b + 1) * 128, :],
                    in_=gbuf[:, b * D : (b + 1) * D],
                )
            nc.sync.dma_start(out=hbufo2[:], in_=hbuf[:, 4 : 4 * (nsteps + 1)])
    nc.compile()
    return nc


# --------------------------------------------------------------------------
# entry point
# --------------------------------------------------------------------------

_exec_times = []


def kernel(hs, ht, c_hs, c_ht, src, dst, W1, b1, W2, b2, W_ih, W_hh, b_ih, b_hh):
    global _exec_times
    _exec_times = []
    hs = np.asarray(hs, dtype=np.float32)
    ht = np.asarray(ht, dtype=np.float32)
    c_hs = np.asarray(c_hs, dtype=np.float32)
    c_ht = np.asarray(c_ht, dtype=np.float32)
    W1 = np.asarray(W1, dtype=np.float32)
    W2 = np.asarray(W2, dtype=np.float32)
    b1 = np.asarray(b1, dtype=np.float32)
    b2 = np.asarray(b2, dtype=np.float32)
    W_ih = np.asarray(W_ih, dtype=np.float32)
    W_hh = np.asarray(W_hh, dtype=np.float32)
    bias = (np.asarray(b_ih, dtype=np.float32) + np.asarray(b_hh, dtype=np.float32))
    assert np.all(b1 == 0) and np.all(b2 == 0), "nonzero GraphConv bias unsupported"

    g = _prep_graph(src, dst)
    x_cat = np.concatenate([ht, c_hs], axis=1)
    x1e_cores = _expand_l1(g, x_cat)

    # LSTM host layouts
    gperm = np.r_[0:2 * OT, 3 * OT : 4 * OT, 2 * OT : 3 * OT]  # (i,f,g,o)->(i,f,o,g)
    wihT = np.ascontiguousarray(W_ih.T[:, gperm])  # [128 in, 512 gates]
    whhT = np.ascontiguousarray(W_hh.T[:, gperm]).astype(np.float16)
    biastT = np.ascontiguousarray(bias[gperm].reshape(4, OT).T)  # [128, 4]
    w2d = np.concatenate([W2, W2], axis=0)  # [128, 64] duplicated halves
    idm = np.eye(128, dtype=np.float16)
    xt_cores = []
    for c in range(NC):
        x = np.concatenate(
            [hs[c * BPC : (c + 1) * BPC], c_ht[c * BPC : (c + 1) * BPC]], axis=2
        )  # [4, 512, 128]
        xt_cores.append(np.ascontiguousarray(x.transpose(2, 1, 0).reshape(F, T * BPC)))

    nc1 = _build_l1(g)
    in_maps1 = [
        {
            "x1e": x1e_cores[c],
            "xt": xt_cores[c],
            "wih": wihT,
            "whh": whhT,
            "biast": biastT,
            "w1": W1,
            "nin": g["nin_core"][c],
            "idm": idm,
        }
        for c in range(NC)
    ]
    res1 = bass_utils.run_bass_kernel_spmd(nc1, in_maps1, core_ids=list(range(NC)))
    if res1.exec_time_ns:
        _exec_times.append(res1.exec_time_ns)

    out1_g = np.zeros((N, D), dtype=np.float32)
    for c in range(NC):
        out1_g[g["perms"][c]] = res1.results[c]["out1"][:NPC]
    x2e_cores = _expand_l2(g, out1_g)

    nc2 = _build_l2(g)
    in_maps2 = [
        {
            "x2e": x2e_cores[c],
            "xt": xt_cores[c],
            "wih": wihT,
            "whh": whhT,
            "biast": biastT,
            "w2d": w2d,
            "nin": g["nin_core"][c],
            "idm": idm,
            "hin": res1.results[c]["ho"],
            "cin": res1.results[c]["co"],
        }
        for c in range(NC)
    ]
    res2 = bass_utils.run_bass_kernel_spmd(nc2, in_maps2, core_ids=list(range(NC)))
    if res2.exec_time_ns:
        _exec_times.append(res2.exec_time_ns)

    x_out = np.zeros((N, D), dtype=np.float32)
    for c in range(NC):
        x_out[g["perms"][c]] = res2.results[c]["out2"][:NPC]

    hs_out = np.zeros((B, T, OT), dtype=np.float32)
    for c in range(NC):
        ha = (
            res1.results[c]["hbufo"].astype(np.float32)
            .reshape(OT, TSPLIT, BPC)
            .transpose(2, 1, 0)
        )
        hb = (
            res2.results[c]["hbufo2"].astype(np.float32)
            .reshape(OT, T - TSPLIT, BPC)
            .transpose(2, 1, 0)
        )
        hs_out[c * BPC : (c + 1) * BPC, :TSPLIT] = ha
        hs_out[c * BPC : (c + 1) * BPC, TSPLIT:] = hb
    return hs_out, x_out
